# revision 1
# baseline (speedup 1.0000x reference)
"""MQA (GQA with 1 KV group) attention kernel for 8 Trainium2 NeuronCores.

Sharding: core c -> batch b = c//4, head-group hg = c%4 (4 of 16 query heads).
Each core computes Q/K/V projections from x[b]^T, causal attention for its 4
heads in transposed layout (S^T[kv, q] tiles), and a partial output
projection out_partial = A_h @ Wo[:, cols_h]^T.  Host sums the 4 partials per
batch and adds bo.

Matmul operands are bf16 (PSUM accumulation is f32); f32 matmuls lower to two
PE passes on trn2, so bf16 halves tensor-engine time and input DMA.  The
causal mask is hardcoded structurally (upper-triangular tiles skipped; 4
precomputed diagonal-block masks), the padding mask enters as a per-kv-
partition bias fused into the exp activation.  Softmax row-sums accumulate on
the PE via a ones-column matmul into a dedicated PSUM bank.
"""

import sys

sys.path.insert(0, "/opt/trn_rl_repo")

import ml_dtypes
import numpy as np

import concourse.bass as bass
import concourse.tile as tile
from concourse import bacc
from concourse import mybir
from concourse.bass import ts
from concourse.bass_utils import run_bass_kernel_spmd
from concourse.masks import make_identity

B, S, HID = 2, 2048, 2048
H, D = 16, 128
HPC = 4              # heads per core
DPH = HPC * D        # 512: head dims per core
NCORES = 8
SC = 512             # s-chunk (free dim for most matmuls)
NSC = S // SC        # 4
NT = S // 128        # 16 128-tiles along s / hid
NHT = HID // 128     # 16 hid tiles
SCALE = 1.0 / float(np.sqrt(D))
NEG = -1.0e9

F32 = mybir.dt.float32
BF16 = mybir.dt.bfloat16
NP_BF16 = ml_dtypes.bfloat16

_PROGRAM = None
LAST_RESULT = None


def _build_program():
    nc = bacc.Bacc()
    xT = nc.declare_dram_parameter("xT", [HID, S], BF16, isOutput=False)
    wq = nc.declare_dram_parameter("wq", [HID, DPH], BF16, isOutput=False)
    wk = nc.declare_dram_parameter("wk", [HID, D], BF16, isOutput=False)
    wv = nc.declare_dram_parameter("wv", [HID, D], BF16, isOutput=False)
    wo = nc.declare_dram_parameter("wo", [DPH, HID], BF16, isOutput=False)
    bq = nc.declare_dram_parameter("bq", [128, HPC], F32, isOutput=False)
    bkv = nc.declare_dram_parameter("bkv", [128, 2], F32, isOutput=False)
    padb = nc.declare_dram_parameter("padb", [128, NT], F32, isOutput=False)
    dmask = nc.declare_dram_parameter("dmask", [128, 4 * SC], F32, isOutput=False)
    out = nc.declare_dram_parameter("out", [S, HID], F32, isOutput=True)

    Exp = mybir.ActivationFunctionType.Exp
    Ident = mybir.ActivationFunctionType.Identity

    with tile.TileContext(nc) as tc:
        with (
            tc.tile_pool(name="consts", bufs=1) as consts,
            tc.tile_pool(name="persist", bufs=1) as persist,
        ):
            ident = consts.tile([128, 128], BF16)
            make_identity(nc, ident[:])
            ones_col = consts.tile([128, 1], F32)
            nc.vector.memset(ones_col[:], 1.0)
            ones_row = consts.tile([1, 128], F32)
            nc.vector.memset(ones_row[:], 1.0)
            bq_sb = consts.tile([128, HPC], F32)
            nc.sync.dma_start(bq_sb[:], bq[:])
            bkv_sb = consts.tile([128, 2], F32)
            nc.sync.dma_start(bkv_sb[:], bkv[:])
            padb_sb = consts.tile([128, NT], F32)
            nc.sync.dma_start(padb_sb[:], padb[:])
            dmask_sb = consts.tile([128, 4 * SC], F32)
            nc.sync.dma_start(dmask_sb[:], dmask[:])

            # Persistent activations (live across stages)
            QT = persist.tile([128, HPC, S], BF16)   # Q^T per head: [d, h, q]
            KT = persist.tile([128, S], BF16)        # K^T: [d, kv]
            V = persist.tile([128, NT, 128], BF16)   # V tiles: [kv_p, kv_tile, d]
            OT = persist.tile([128, HPC, S], BF16)   # (exp(S) V)^T scaled

            # ---------------- Stage 1: projections ----------------
            with (
                tc.tile_pool(name="w1", bufs=1) as w1p,
                tc.tile_pool(name="xt", bufs=20) as xtp,
                tc.tile_pool(name="vt", bufs=2) as vtp,
                tc.tile_pool(name="ps1", bufs=1, space="PSUM") as ps1,
                tc.tile_pool(name="pstr", bufs=2, space="PSUM") as pstr,
            ):
                wq_sb = w1p.tile([128, NHT, DPH], BF16)
                nc.sync.dma_start(
                    wq_sb[:], wq.rearrange("(t p) d -> p t d", p=128)
                )
                wk_sb = w1p.tile([128, NHT, D], BF16)
                nc.sync.dma_start(
                    wk_sb[:], wk.rearrange("(t p) d -> p t d", p=128)
                )
                wv_sb = w1p.tile([128, NHT, D], BF16)
                nc.sync.dma_start(
                    wv_sb[:], wv.rearrange("(t p) d -> p t d", p=128)
                )

                for sc in range(NSC):
                    xts = []
                    for ht in range(NHT):
                        xt_t = xtp.tile([128, SC], BF16, tag="xt")
                        nc.sync.dma_start(
                            xt_t[:], xT[ts(ht, 128), ts(sc, SC)]
                        )
                        xts.append(xt_t)
                    # K^T chunk
                    psk = ps1.tile([128, SC], F32, tag="k")
                    for ht in range(NHT):
                        nc.tensor.matmul(
                            psk[:], wk_sb[:, ht, :], xts[ht][:],
                            start=(ht == 0), stop=(ht == NHT - 1),
                        )
                    nc.scalar.activation(
                        KT[:, ts(sc, SC)], psk[:], Ident, bias=bkv_sb[:, 0:1]
                    )
                    # V^T chunk -> transpose into V tiles
                    psv = ps1.tile([128, SC], F32, tag="v")
                    for ht in range(NHT):
                        nc.tensor.matmul(
                            psv[:], wv_sb[:, ht, :], xts[ht][:],
                            start=(ht == 0), stop=(ht == NHT - 1),
                        )
                    vt_s = vtp.tile([128, SC], BF16, tag="vt")
                    nc.scalar.activation(
                        vt_s[:], psv[:], Ident, bias=bkv_sb[:, 1:2]
                    )
                    for j in range(SC // 128):
                        pst = pstr.tile([128, 128], BF16, tag="tr")
                        nc.tensor.transpose(pst[:], vt_s[:, ts(j, 128)], ident[:])
                        nc.scalar.copy(V[:, sc * 4 + j, :], pst[:])
                    # Q^T chunks (4 heads)
                    for dt in range(HPC):
                        psq = ps1.tile([128, SC], F32, tag=f"q{dt}")
                        for ht in range(NHT):
                            nc.tensor.matmul(
                                psq[:], wq_sb[:, ht, ts(dt, 128)], xts[ht][:],
                                start=(ht == 0), stop=(ht == NHT - 1),
                            )
                        nc.scalar.activation(
                            QT[:, dt, ts(sc, SC)], psq[:], Ident,
                            bias=bq_sb[:, dt : dt + 1],
                        )

            # ---------------- Stage 2: attention ----------------
            with (
                tc.tile_pool(name="wo", bufs=1) as wop,
                tc.tile_pool(name="es", bufs=8) as esp,
                tc.tile_pool(name="acc", bufs=2) as accp,
                tc.tile_pool(name="rs", bufs=2) as rsp,
            ):
                wo_sb = wop.tile([128, HPC, HID], BF16)
                nc.sync.dma_start(
                    wo_sb[:], wo.rearrange("(t p) d -> p t d", p=128)
                )

                with (
                    tc.tile_pool(name="psS", bufs=2, space="PSUM") as psS,
                    tc.tile_pool(name="psO", bufs=1, space="PSUM") as psO,
                    tc.tile_pool(name="psR", bufs=1, space="PSUM") as psR,
                    tc.tile_pool(name="psB", bufs=1, space="PSUM") as psB,
                ):
                  for qc in range(NSC):
                    nkt = 4 * qc + 4
                    psos = [
                        psO.tile([128, SC], F32, tag=f"o{h}", name=f"pso_{h}")
                        for h in range(HPC)
                    ]
                    accs = [
                        accp.tile([128, SC], F32, tag=f"a{h}", name=f"acc_{h}")
                        for h in range(HPC)
                    ]
                    for h in range(HPC):
                        nc.vector.memset(accs[h][:], 0.0)
                    for kt in range(nkt):
                        ess = []
                        for h in range(HPC):
                            ps = psS.tile([128, SC], F32, tag="s")
                            nc.tensor.matmul(
                                ps[:], KT[:, ts(kt, 128)], QT[:, h, ts(qc, SC)],
                                start=True, stop=True,
                            )
                            j = kt - 4 * qc
                            if j >= 0:
                                nc.vector.tensor_add(
                                    ps[:], ps[:], dmask_sb[:, ts(j, SC)]
                                )
                            es = esp.tile([128, SC], BF16, tag="es")
                            nc.scalar.activation(
                                es[:], ps[:], Exp,
                                bias=padb_sb[:, kt : kt + 1], scale=SCALE,
                            )
                            ess.append(es)
                        for h in range(HPC):
                            nc.tensor.matmul(
                                psos[h][:], V[:, kt, :], ess[h][:],
                                start=(kt == 0), stop=(kt == nkt - 1),
                            )
                            nc.vector.tensor_add(
                                accs[h][:], accs[h][:], ess[h][:]
                            )
                    for h in range(HPC):
                        psr_t = psR.tile([1, SC], F32, tag="r")
                        nc.tensor.matmul(
                            psr_t[:], ones_col[:], accs[h][:],
                            start=True, stop=True,
                        )
                        rs = rsp.tile([1, SC], F32, tag="rs")
                        nc.vector.reciprocal(rs[:], psr_t[:])
                        psb = psB.tile([128, SC], F32, tag="b")
                        nc.tensor.matmul(
                            psb[:], ones_row[:], rs[:], start=True, stop=True
                        )
                        bb = rsp.tile([128, SC], F32, tag="bb")
                        nc.scalar.copy(bb[:], psb[:])
                        nc.vector.tensor_mul(
                            OT[:, h, ts(qc, SC)], psos[h][:], bb[:]
                        )

                # ---------------- Stage 3: output projection ----------------
                with (
                    tc.tile_pool(name="outsb", bufs=4) as outp,
                    tc.tile_pool(name="ps3", bufs=1, space="PSUM") as ps3,
                ):
                    for st in range(NT):
                        pss = [
                            ps3.tile([128, SC], F32, tag=f"c{hc}", name=f"ps3_{hc}")
                            for hc in range(HID // SC)
                        ]
                        for dt in range(HPC):
                            for hc in range(HID // SC):
                                nc.tensor.matmul(
                                    pss[hc][:],
                                    OT[:, dt, ts(st, 128)],
                                    wo_sb[:, dt, ts(hc, SC)],
                                    start=(dt == 0), stop=(dt == HPC - 1),
                                )
                        for hc in range(HID // SC):
                            ot = outp.tile([128, SC], F32, tag="out")
                            nc.scalar.copy(ot[:], pss[hc][:])
                            nc.sync.dma_start(
                                out[ts(st, 128), ts(hc, SC)], ot[:]
                            )
    nc.compile()
    return nc


def _get_program():
    global _PROGRAM
    if _PROGRAM is None:
        _PROGRAM = _build_program()
    return _PROGRAM


def kernel(**inputs):
    global LAST_RESULT
    hs = np.ascontiguousarray(inputs["hidden_states"], dtype=np.float32)
    pad = np.ascontiguousarray(inputs["padding_mask"], dtype=np.float32)
    Wq = np.asarray(inputs["Wq"], dtype=np.float32)
    Wk = np.asarray(inputs["Wk"], dtype=np.float32)
    Wv = np.asarray(inputs["Wv"], dtype=np.float32)
    Wo = np.asarray(inputs["Wo"], dtype=np.float32)
    bq_v = np.asarray(inputs["bq"], dtype=np.float32)
    bk_v = np.asarray(inputs["bk"], dtype=np.float32)
    bv_v = np.asarray(inputs["bv"], dtype=np.float32)
    bo_v = np.asarray(inputs["bo"], dtype=np.float32)

    xTs = [np.ascontiguousarray(hs[b].T).astype(NP_BF16) for b in range(B)]
    WqT = Wq.T  # [HID, HID]
    WkT = np.ascontiguousarray(Wk.T).astype(NP_BF16)  # [HID, D]
    WvT = np.ascontiguousarray(Wv.T).astype(NP_BF16)
    WoT = Wo.T  # [HID, HID]

    # 4 diagonal-block masks in S^T layout: mask_j[p, f] = 0 if p+128*j <= f
    p_i = np.arange(128)[:, None]
    f_i = np.arange(SC)[None, :]
    dmask = np.empty((128, 4 * SC), np.float32)
    for j in range(4):
        dmask[:, j * SC : (j + 1) * SC] = np.where(
            p_i + 128 * j <= f_i, 0.0, NEG
        ).astype(np.float32)

    padbs = [
        np.ascontiguousarray((NEG * pad[b]).reshape(NT, 128).T) for b in range(B)
    ]
    bqs = [
        np.ascontiguousarray(
            bq_v[hg * DPH : (hg + 1) * DPH].reshape(HPC, 128).T
        )
        for hg in range(HPC)
    ]
    bkv = np.ascontiguousarray(np.stack([bk_v, bv_v], axis=1))  # [128, 2]

    nc = _get_program()
    in_maps = []
    for c in range(NCORES):
        b, hg = c // 4, c % 4
        in_maps.append(
            {
                "xT": xTs[b],
                "wq": np.ascontiguousarray(
                    WqT[:, hg * DPH : (hg + 1) * DPH]
                ).astype(NP_BF16),
                "wk": WkT,
                "wv": WvT,
                "wo": np.ascontiguousarray(
                    WoT[hg * DPH : (hg + 1) * DPH, :]
                ).astype(NP_BF16),
                "bq": bqs[hg],
                "bkv": bkv,
                "padb": padbs[b],
                "dmask": dmask,
            }
        )

    LAST_RESULT = run_bass_kernel_spmd(nc, in_maps, list(range(NCORES)))
    res = LAST_RESULT.results

    outp = np.zeros((B, S, HID), np.float32)
    for c in range(NCORES):
        outp[c // 4] += res[c]["out"]
    outp += bo_v[None, None, :]
    return outp


if __name__ == "__main__":
    rng = np.random.default_rng(0)
    demo = {
        "hidden_states": rng.standard_normal((B, S, HID), dtype=np.float32),
        "causal_mask": np.triu(np.ones((1, 1, S, S), np.float32), k=1),
        "padding_mask": np.zeros((B, S), np.float32),
        "Wq": (rng.standard_normal((HID, HID), dtype=np.float32) * 0.02),
        "bq": np.zeros((HID,), np.float32),
        "Wk": (rng.standard_normal((D, HID), dtype=np.float32) * 0.02),
        "bk": np.zeros((D,), np.float32),
        "Wv": (rng.standard_normal((D, HID), dtype=np.float32) * 0.02),
        "bv": np.zeros((D,), np.float32),
        "Wo": (rng.standard_normal((HID, HID), dtype=np.float32) * 0.02),
        "bo": np.zeros((HID,), np.float32),
    }
    o = kernel(**demo)
    print("kernel output", o.shape, o.dtype, float(np.abs(o).mean()))



# revision 8
# speedup vs baseline: 1.5582x; 1.5582x over previous
"""MQA (GQA with 1 KV group) attention kernel for 8 Trainium2 NeuronCores.

Sharding: core c -> batch b = c//4, head-group hg = c%4 (4 of 16 query heads).
Each core computes Q/K/V projections from x[b]^T, causal attention for its 4
heads in transposed layout (S^T[kv, q] tiles), and a partial output
projection out_partial = A_h @ Wo[:, cols_h]^T.  Host sums the 4 bf16
partials per batch and adds bo.

v2 notes vs v1:
- softmax row-sums accumulate on the PE: per (kt, h) an M=1 matmul with a
  ones column adds exp-tile column sums into one PSUM bank at partitions
  {0,32,64,96} (col-tiled via tile_position so the 4 heads' row-sum matmuls
  can run concurrently in distinct 32-col array groups).
- causal masking is multiplicative-after-exp on the [128,128] diagonal block
  only; fully-masked columns of diagonal k-tiles are never computed (matmul
  and activation column ranges trimmed).
- reciprocal via reciprocal_approx_fast on a [128,512] tile (one per qc).
- PSUM->SBUF evacuations on the vector engine (DVE), not scalar.
- inputs host-packed partition-contiguous for wide DMA lines; output partials
  are bf16.
"""

import sys

sys.path.insert(0, "/opt/trn_rl_repo")

import ml_dtypes
import numpy as np

import concourse.bass as bass
import concourse.tile as tile
from concourse import bacc
from concourse import mybir
from concourse.bass import ts
from concourse.bass_utils import run_bass_kernel_spmd
from concourse.masks import make_identity

B, S, HID = 2, 2048, 2048
H, D = 16, 128
HPC = 4              # heads per core
DPH = HPC * D        # 512: head dims per core
NCORES = 8
SC = 512             # s-chunk (free dim for most matmuls)
NSC = S // SC        # 4
NT = S // 128        # 16 128-tiles along s / hid
NHT = HID // 128     # 16 hid tiles
SCALE = 1.0 / float(np.sqrt(D))
NEG = -1.0e9

F32 = mybir.dt.float32
BF16 = mybir.dt.bfloat16
NP_BF16 = ml_dtypes.bfloat16

_PROGRAM = None
LAST_RESULT = None


def _build_program():
    nc = bacc.Bacc()
    xTp = nc.declare_dram_parameter("xTp", [128, NSC, NHT, SC], BF16, isOutput=False)
    wq = nc.declare_dram_parameter("wq", [128, NHT, DPH], BF16, isOutput=False)
    wk = nc.declare_dram_parameter("wk", [128, NHT, D], BF16, isOutput=False)
    wv = nc.declare_dram_parameter("wv", [128, NHT, D], BF16, isOutput=False)
    wo = nc.declare_dram_parameter("wo", [128, HPC, HID], BF16, isOutput=False)
    bq = nc.declare_dram_parameter("bq", [128, HPC], F32, isOutput=False)
    bkv = nc.declare_dram_parameter("bkv", [128, 2], F32, isOutput=False)
    padb = nc.declare_dram_parameter("padb", [128, NT], F32, isOutput=False)
    ltm = nc.declare_dram_parameter("ltm", [128, 128], BF16, isOutput=False)
    out = nc.declare_dram_parameter("out", [128, NT, HID], BF16, isOutput=True)

    Exp = mybir.ActivationFunctionType.Exp
    Ident = mybir.ActivationFunctionType.Identity

    with tile.TileContext(nc) as tc:
        with (
            tc.tile_pool(name="consts", bufs=1) as consts,
            tc.tile_pool(name="weights", bufs=1) as wpool,
            tc.tile_pool(name="persist", bufs=1) as persist,
        ):
            ident = consts.tile([128, 128], BF16)
            make_identity(nc, ident[:])
            ones_sq = consts.tile([128, 128], BF16)
            nc.vector.memset(ones_sq[:], 1.0)
            bq_sb = consts.tile([128, HPC], F32)
            nc.sync.dma_start(bq_sb[:], bq[:])
            bkv_sb = consts.tile([128, 2], F32)
            nc.sync.dma_start(bkv_sb[:], bkv[:])
            padb_sb = consts.tile([128, NT], F32)
            nc.sync.dma_start(padb_sb[:], padb[:])
            ltm_sb = consts.tile([128, 128], BF16)
            nc.sync.dma_start(ltm_sb[:], ltm[:])

            wk_sb = wpool.tile([128, NHT, D], BF16)
            nc.sync.dma_start(wk_sb[:], wk[:])
            wv_sb = wpool.tile([128, NHT, D], BF16)
            nc.sync.dma_start(wv_sb[:], wv[:])
            wq_sb = wpool.tile([128, NHT, DPH], BF16)
            wo_sb = wpool.tile([128, HPC, HID], BF16)

            # Persistent activations (live across stages)
            QT = persist.tile([128, HPC, S], BF16)   # Q^T per head: [d, h, q]
            KT = persist.tile([128, S], BF16)        # K^T: [d, kv]
            V = persist.tile([128, NT, 128], BF16)   # V tiles: [kv_p, kv_tile, d]
            OT = persist.tile([128, HPC, S], BF16)   # softmax(S) V, transposed

            # ---------------- Stage 1: projections ----------------
            with (
                tc.tile_pool(name="xt", bufs=2) as xtp,
                tc.tile_pool(name="vt", bufs=2) as vtp,
                tc.tile_pool(name="ps1", bufs=1, space="PSUM") as ps1,
                tc.tile_pool(name="pstr", bufs=2, space="PSUM") as pstr,
            ):
                for sc in range(NSC):
                    xt = xtp.tile([128, NHT, SC], BF16, tag="xt")
                    nc.sync.dma_start(xt[:], xTp[:, sc])
                    if sc == 0:
                        # big weight loads queue behind the first x chunk
                        nc.sync.dma_start(wq_sb[:], wq[:])
                    # K^T chunk
                    psk = ps1.tile([128, SC], F32, tag="k")
                    for ht in range(NHT):
                        nc.tensor.matmul(
                            psk[:], wk_sb[:, ht, :], xt[:, ht, :],
                            start=(ht == 0), stop=(ht == NHT - 1),
                        )
                    nc.scalar.activation(
                        KT[:, ts(sc, SC)], psk[:], Ident, bias=bkv_sb[:, 0:1]
                    )
                    # V^T chunk -> transpose into V tiles
                    psv = ps1.tile([128, SC], F32, tag="v")
                    for ht in range(NHT):
                        nc.tensor.matmul(
                            psv[:], wv_sb[:, ht, :], xt[:, ht, :],
                            start=(ht == 0), stop=(ht == NHT - 1),
                        )
                    vt_s = vtp.tile([128, SC], BF16, tag="vt")
                    nc.scalar.activation(
                        vt_s[:], psv[:], Ident, bias=bkv_sb[:, 1:2]
                    )
                    for jj in range(SC // 128):
                        pst = pstr.tile([128, 128], BF16, tag="tr")
                        nc.tensor.transpose(pst[:], vt_s[:, ts(jj, 128)], ident[:])
                        nc.vector.tensor_copy(V[:, sc * 4 + jj, :], pst[:])
                    # Q^T chunks (4 heads)
                    for dt in range(HPC):
                        psq = ps1.tile([128, SC], F32, tag=f"q{dt}")
                        for ht in range(NHT):
                            nc.tensor.matmul(
                                psq[:], wq_sb[:, ht, ts(dt, 128)], xt[:, ht, :],
                                start=(ht == 0), stop=(ht == NHT - 1),
                            )
                        nc.scalar.activation(
                            QT[:, dt, ts(sc, SC)], psq[:], Ident,
                            bias=bq_sb[:, dt : dt + 1],
                        )

            # ---------------- Stage 2: attention ----------------
            with (
                tc.tile_pool(name="es", bufs=4) as esp,
                tc.tile_pool(name="rsp", bufs=2) as rsp,
                tc.tile_pool(name="psS", bufs=3, space="PSUM") as psS,
                tc.tile_pool(name="psO", bufs=1, space="PSUM") as psO,
                tc.tile_pool(name="psR", bufs=1, space="PSUM") as psR,
            ):
                nc.sync.dma_start(wo_sb[:], wo[:])
                for qc in range(NSC):
                    nkt = 4 * qc + 4
                    # 2 heads per pass: PSUM budget is 3 (psS) + 2 (psO) + 2 (psR)
                    for hp in range(HPC // 2):
                        heads = (2 * hp, 2 * hp + 1)
                        psos = [
                            psO.tile([128, SC], F32, tag=f"o{i}", name=f"pso_{i}")
                            for i in range(2)
                        ]
                        # row-sums via ones-matrix matmul: every output partition
                        # gets the same column sum -> pre-broadcast row-sums
                        psrb = [
                            psR.tile([128, SC], F32, tag=f"r{i}", name=f"psr_{i}")
                            for i in range(2)
                        ]
                        for kt in range(nkt):
                            j = kt - 4 * qc
                            off = 128 * j if j >= 0 else 0
                            w = SC - off
                            ess = []
                            for i, h in enumerate(heads):
                                ps = psS.tile([128, SC], F32, tag="s")
                                nc.tensor.matmul(
                                    ps[:, 0:w], KT[:, ts(kt, 128)],
                                    QT[:, h, qc * SC + off : (qc + 1) * SC],
                                    start=True, stop=True,
                                )
                                es = esp.tile([128, SC], BF16, tag="es")
                                nc.scalar.activation(
                                    es[:, 0:w], ps[:, 0:w], Exp,
                                    bias=padb_sb[:, kt : kt + 1], scale=SCALE,
                                )
                                if j >= 0:
                                    # zero the strictly-lower (kv > q) part of the
                                    # diagonal 128-block (local cols 0:128)
                                    nc.vector.tensor_mul(
                                        es[:, 0:128], es[:, 0:128], ltm_sb[:]
                                    )
                                ess.append(es)
                            for i in range(2):
                                nc.tensor.matmul(
                                    psos[i][:, off:SC], V[:, kt, :],
                                    ess[i][:, 0:w],
                                    start=(kt == 0), stop=(kt == nkt - 1),
                                )
                                nc.tensor.matmul(
                                    psrb[i][:, off:SC], ones_sq[:],
                                    ess[i][:, 0:w],
                                    start=(kt == 0), stop=(kt == nkt - 1),
                                )
                        # normalization tail for this pass
                        for i, h in enumerate(heads):
                            rs = rsp.tile([128, SC], F32, tag="rs")
                            nc.vector.tensor_copy(rs[:], psrb[i][:])
                            bb = rsp.tile([128, SC], F32, tag="bb")
                            nc.vector.reciprocal_approx_fast(bb[:], rs[:])
                            nc.vector.tensor_mul(
                                OT[:, h, ts(qc, SC)], psos[i][:], bb[:]
                            )

            # ---------------- Stage 3: output projection ----------------
            with (
                tc.tile_pool(name="outsb", bufs=3) as outp,
                tc.tile_pool(name="ps3", bufs=2, space="PSUM") as ps3,
            ):
                for st in range(NT):
                    pss = [
                        ps3.tile([128, SC], F32, tag=f"c{hc}", name=f"ps3_{hc}")
                        for hc in range(HID // SC)
                    ]
                    for dt in range(HPC):
                        for hc in range(HID // SC):
                            nc.tensor.matmul(
                                pss[hc][:],
                                OT[:, dt, ts(st, 128)],
                                wo_sb[:, dt, ts(hc, SC)],
                                start=(dt == 0), stop=(dt == HPC - 1),
                            )
                    ot = outp.tile([128, HID], BF16, tag="ot")
                    for hc in range(HID // SC):
                        nc.vector.tensor_copy(ot[:, ts(hc, SC)], pss[hc][:])
                    nc.sync.dma_start(out[:, st, :], ot[:])
    nc.compile()
    return nc


def _get_program():
    global _PROGRAM
    if _PROGRAM is None:
        _PROGRAM = _build_program()
    return _PROGRAM


def _pack_pt(a, p=128):
    """[T*p, N] -> [p, T, N] partition-contiguous."""
    t = a.shape[0] // p
    return np.ascontiguousarray(a.reshape(t, p, *a.shape[1:]).transpose(1, 0, 2))


def kernel(**inputs):
    global LAST_RESULT
    hs = np.ascontiguousarray(inputs["hidden_states"], dtype=np.float32)
    pad = np.ascontiguousarray(inputs["padding_mask"], dtype=np.float32)
    Wq = np.asarray(inputs["Wq"], dtype=np.float32)
    Wk = np.asarray(inputs["Wk"], dtype=np.float32)
    Wv = np.asarray(inputs["Wv"], dtype=np.float32)
    Wo = np.asarray(inputs["Wo"], dtype=np.float32)
    bq_v = np.asarray(inputs["bq"], dtype=np.float32)
    bk_v = np.asarray(inputs["bk"], dtype=np.float32)
    bv_v = np.asarray(inputs["bv"], dtype=np.float32)
    bo_v = np.asarray(inputs["bo"], dtype=np.float32)

    # x[b]^T packed [128, NSC, NHT, SC]: partition p, s-chunk, hid-tile, s'
    xTps = []
    for b in range(B):
        xT = hs[b].T.astype(NP_BF16)          # [HID, S]
        xTps.append(
            np.ascontiguousarray(
                xT.reshape(NHT, 128, NSC, SC).transpose(1, 2, 0, 3)
            )
        )
    WqT = Wq.T  # [HID, HID]
    wk_p = _pack_pt(np.ascontiguousarray(Wk.T).astype(NP_BF16))   # [128,16,128]
    wv_p = _pack_pt(np.ascontiguousarray(Wv.T).astype(NP_BF16))
    WoT = Wo.T  # [HID, HID]

    ltm = np.triu(np.ones((128, 128), np.float32)).astype(NP_BF16)

    padbs = [
        np.ascontiguousarray((NEG * pad[b]).reshape(NT, 128).T) for b in range(B)
    ]
    bqs = [
        np.ascontiguousarray(
            bq_v[hg * DPH : (hg + 1) * DPH].reshape(HPC, 128).T
        )
        for hg in range(HPC)
    ]
    bkv = np.ascontiguousarray(np.stack([bk_v, bv_v], axis=1))  # [128, 2]

    wq_ps = [
        _pack_pt(
            np.ascontiguousarray(WqT[:, hg * DPH : (hg + 1) * DPH]).astype(NP_BF16)
        )
        for hg in range(HPC)
    ]
    wo_ps = [
        _pack_pt(
            np.ascontiguousarray(WoT[hg * DPH : (hg + 1) * DPH, :]).astype(NP_BF16)
        )
        for hg in range(HPC)
    ]

    nc = _get_program()
    in_maps = []
    for c in range(NCORES):
        b, hg = c // 4, c % 4
        in_maps.append(
            {
                "xTp": xTps[b],
                "wq": wq_ps[hg],
                "wk": wk_p,
                "wv": wv_p,
                "wo": wo_ps[hg],
                "bq": bqs[hg],
                "bkv": bkv,
                "padb": padbs[b],
                "ltm": ltm,
            }
        )

    LAST_RESULT = run_bass_kernel_spmd(nc, in_maps, list(range(NCORES)))
    res = LAST_RESULT.results

    outp = np.zeros((B, S, HID), np.float32)
    for c in range(NCORES):
        part = np.asarray(res[c]["out"], dtype=np.float32)  # [128, NT, HID]
        outp[c // 4] += part.transpose(1, 0, 2).reshape(S, HID)
    outp += bo_v[None, None, :]
    return outp


if __name__ == "__main__":
    rng = np.random.default_rng(0)
    demo = {
        "hidden_states": rng.standard_normal((B, S, HID), dtype=np.float32),
        "causal_mask": np.triu(np.ones((1, 1, S, S), np.float32), k=1),
        "padding_mask": np.zeros((B, S), np.float32),
        "Wq": (rng.standard_normal((HID, HID), dtype=np.float32) * 0.02),
        "bq": np.zeros((HID,), np.float32),
        "Wk": (rng.standard_normal((D, HID), dtype=np.float32) * 0.02),
        "bk": np.zeros((D,), np.float32),
        "Wv": (rng.standard_normal((D, HID), dtype=np.float32) * 0.02),
        "bv": np.zeros((D,), np.float32),
        "Wo": (rng.standard_normal((HID, HID), dtype=np.float32) * 0.02),
        "bo": np.zeros((HID,), np.float32),
    }
    o = kernel(**demo)
    print("kernel output", o.shape, o.dtype, float(np.abs(o).mean()))
